# revision 10
# baseline (speedup 1.0000x reference)
"""Trainium2 Bass kernel for nn_MaskToken (scatter_memory).

Reference semantics (B=4, L=4096, D=1024, NUM_KEEP=1024):
  outputs_dropped[b, k, :] = inputs[b, idx_keep[k], :]          # gather
  outputs_masked[b, l, :]  = inputs[b, l, :] if l in idx_keep else mask_embedding
  mask_drop[l]             = 0.0 if l in idx_keep else 1.0
  idx_keep passthrough.

Strategy: shard the length axis across 8 cores (512 rows/core, all 4
batches -> 2048 flat rows of 4KB each per core). idx_keep is sorted, so
the kept rows of core c map to a contiguous span of outputs_dropped —
no collective needed. The device program is pure DMA data movement:

  1. one bulk dma_gather pulls the core's kept rows (HBM -> SBUF);
     token i lands at SBUF slot (i % 128, i // 128)
  2. one bulk dma_scatter_add pushes those rows into the zero-initialized
     outputs_dropped at packed positions (add-to-zero == write)
  3. HWDGE fills every outputs_masked row with mask_embedding (static
     pattern), then 16 indirect-DMA copy-scatters overwrite the kept
     rows with the gathered data (copy, not add, so the fill must not
     be added to)

Per-core counts are data-dependent; the single SPMD program takes the
real token count in a register (loaded from a tiny input tensor), pads
the int16 token lists with -1 (consumed only up to the count) and pads
the int32 scatter lists with a sentinel the DMA bounds check skips.
"""

import numpy as np

import concourse.bacc as bacc
import concourse.bass as bass
import concourse.mybir as mybir
from concourse.bass_utils import run_bass_kernel_spmd

B, L, D = 4, 4096, 1024
NUM_KEEP = 1024
N_CORES = 8
LS = L // N_CORES            # 512 rows per core
R = B * LS                   # 2048 flat rows per core (b*LS + l_local)
COLS = R // 128              # 16 token columns
SENT = np.int32(8192)        # int32 OOB sentinel (> R-1 -> bounds-check skip)

F32 = mybir.dt.float32
I32 = mybir.dt.int32
I16 = mybir.dt.int16

_BUILT = None


def _build():
    """One SPMD Bass program shared by all 8 cores."""
    nc = bacc.Bacc(None)
    x = nc.dram_tensor("x", [R, D], F32, kind="ExternalInput")
    meb = nc.dram_tensor("meb", [128, D], F32, kind="ExternalInput")
    # i16 token lists (wrapped [16, R/16], replicated to 128 partitions):
    # cols [0:R/16] = gather src rows, cols [R/16:2*R/16] = od dst rows
    idx16 = nc.dram_tensor("idx16", [128, 2 * (R // 16)], I16,
                           kind="ExternalInput")
    # i32 om dst rows for the kept-row copy-scatter, laid out [p, c] for
    # token c*128+p, SENT-padded
    kdst = nc.dram_tensor("kdst", [128, COLS], I32, kind="ExternalInput")
    cnt = nc.dram_tensor("cnt", [1, 1], I32, kind="ExternalInput")
    om = nc.dram_tensor("om", [R, D], F32, kind="ExternalOutput")
    od = nc.dram_tensor("od", [R + 1, D], F32, kind="ExternalOutput")

    W16 = R // 16

    with (
        nc.Block() as block,
        nc.semaphore("ld") as ld,
        nc.semaphore("fl") as fl,
        nc.semaphore("g0") as g0,
        nc.semaphore("sc") as sc,
        nc.sbuf_tensor("xk", [128, COLS, D], F32) as xk,
        nc.sbuf_tensor("mesb", [128, D], F32) as mesb,
        nc.sbuf_tensor("i16sb", [128, 2 * W16], I16) as i16sb,
        nc.sbuf_tensor("kdsb", [128, COLS], I32) as kdsb,
        nc.sbuf_tensor("cntsb", [1, 1], I32) as cntsb,
    ):
        @block.sync
        def _(sync):
            sync.dma_start(out=i16sb[:], in_=idx16[:]).then_inc(ld, 16)
            sync.dma_start(out=kdsb[:], in_=kdst[:]).then_inc(ld, 16)
            sync.dma_start(out=cntsb[:], in_=cnt[:]).then_inc(ld, 16)
            sync.dma_start(out=mesb[:], in_=meb[:]).then_inc(ld, 16)
            sync.wait_ge(ld, 64)
            for j in range(COLS):
                sync.dma_start(
                    out=om[j * 128:(j + 1) * 128, :], in_=mesb[:]
                ).then_inc(fl, 16)

        @block.gpsimd
        def _(g):
            g.wait_ge(ld, 64)
            with g.register("rk") as rk, g.register("bc") as bc:
                g.reg_load(rk, cntsb[:1, :1])
                g.reg_mov(bc, R - 1)
                g.dma_gather(
                    out_ap=xk[:],
                    in_ap=x[:],
                    idxs_ap=i16sb[:, 0:W16],
                    num_idxs=R,
                    num_idxs_reg=rk,
                    elem_size=D,
                ).then_inc(g0, 16)
                g.wait_ge(g0, 16)
                g.dma_scatter_add(
                    out_ap=od[:],
                    in_ap=xk[:],
                    idxs_ap=i16sb[:, W16:2 * W16],
                    num_idxs=R,
                    num_idxs_reg=rk,
                    elem_size=D,
                ).then_inc(sc, 16)
                g.wait_ge(fl, 16 * COLS)
                for c in range(COLS):
                    g.indirect_dma_start(
                        out=om[:],
                        out_offset=bass.IndirectOffsetOnAxis(
                            ap=kdsb[:, c:c + 1], axis=0),
                        in_=xk[:, c:c + 1, :].squeeze(1),
                        in_offset=None,
                        bounds_check=bc,
                        oob_is_err=False,
                    ).then_inc(sc, 16)
                g.wait_ge(sc, 16 * (COLS + 1))

    nc.compile()
    return nc


def get_program():
    global _BUILT
    if _BUILT is None:
        _BUILT = _build()
    return _BUILT


def _wrap16(flat):
    """int16 token list -> [128, len/16] wrapped layout: token i at
    [i % 16, i // 16], replicated 8x along partitions."""
    arr = np.asarray(flat, dtype=np.int16).reshape(-1, 16).T  # [16, n/16]
    return np.tile(arr, (8, 1))


def make_core_inputs(inputs, mask_embedding, idx_keep):
    """Host-side sharding: slice x per core, build token lists."""
    idx_keep = np.asarray(idx_keep)
    keep_starts = np.searchsorted(idx_keep, np.arange(0, L + LS, LS))
    meb = np.broadcast_to(mask_embedding.astype(np.float32), (128, D)).copy()

    in_maps = []
    counts = []
    for c in range(N_CORES):
        lo, hi = c * LS, (c + 1) * LS
        kl = (idx_keep[keep_starts[c]:keep_starts[c + 1]] - lo).astype(np.int64)
        n_c = len(kl)
        nk = B * n_c

        # token i (b-major over kept rows): src row b*LS+kl[j], od dst
        # b*LS+j, om dst == src row
        src = np.full(R, -1, dtype=np.int64)
        ddst = np.full(R, -1, dtype=np.int64)
        for b in range(B):
            src[b * n_c:(b + 1) * n_c] = b * LS + kl
            ddst[b * n_c:(b + 1) * n_c] = b * LS + np.arange(n_c)
        n_tok = nk
        if nk < R:                     # dummy token keeps the count nonzero
            src[nk] = 0
            ddst[nk] = R               # sacrificial od row
            n_tok = nk + 1

        kdst = np.full((128, COLS), SENT, dtype=np.int32)
        i_tok = np.arange(nk)
        kdst[i_tok % 128, i_tok // 128] = src[:nk]   # om dst == src row

        x_c = np.ascontiguousarray(
            inputs[:, lo:hi, :], dtype=np.float32).reshape(R, D)
        in_maps.append({
            "x": x_c,
            "meb": meb,
            "idx16": np.concatenate([_wrap16(src), _wrap16(ddst)], axis=1),
            "kdst": kdst,
            "cnt": np.array([[n_tok]], dtype=np.int32),
        })
        counts.append(n_c)
    return in_maps, counts, keep_starts


def kernel(inputs, mask_embedding, idx_keep):
    inputs = np.asarray(inputs)
    mask_embedding = np.asarray(mask_embedding)
    idx_keep = np.asarray(idx_keep).astype(np.int32)

    nc = get_program()
    in_maps, counts, keep_starts = make_core_inputs(
        inputs, mask_embedding, idx_keep)

    res = run_bass_kernel_spmd(nc, in_maps, list(range(N_CORES)))

    outputs_masked = np.empty((B, L, D), dtype=np.float32)
    outputs_dropped = np.empty((B, NUM_KEEP, D), dtype=np.float32)
    for c in range(N_CORES):
        lo, hi = c * LS, (c + 1) * LS
        outputs_masked[:, lo:hi, :] = res.results[c]["om"].reshape(B, LS, D)
        n_c = counts[c]
        k0 = keep_starts[c]
        outputs_dropped[:, k0:k0 + n_c, :] = (
            res.results[c]["od"][:R].reshape(B, LS, D)[:, :n_c, :])

    mask_drop = np.ones(L, dtype=np.float32)
    mask_drop[idx_keep] = 0.0

    return outputs_dropped, outputs_masked, mask_drop, idx_keep


# revision 11
# speedup vs baseline: 1.1660x; 1.1660x over previous
"""Trainium2 Bass kernel for nn_MaskToken (scatter_memory).

Reference semantics (B=4, L=4096, D=1024, NUM_KEEP=1024):
  outputs_dropped[b, k, :] = inputs[b, idx_keep[k], :]          # gather
  outputs_masked[b, l, :]  = inputs[b, l, :] if l in idx_keep else mask_embedding
  mask_drop[l]             = 0.0 if l in idx_keep else 1.0
  idx_keep passthrough.

Strategy: shard the length axis across 8 cores (512 rows/core). Each
core's slice is laid out length-major ([512, B*D] = 512 groups of 16KB),
so one DMA token moves a length position for all 4 batches at once —
4x fewer descriptors than row-granular movement, which is what the Q7
descriptor-generation cost scales with. idx_keep is sorted, so the kept
rows of core c map to a contiguous span of outputs_dropped — no
collective needed. The device program is pure DMA data movement:

  1. one bulk dma_gather pulls the kept 16KB groups (HBM -> SBUF);
     token j lands at SBUF slot (j % 128, j // 128)
  2. one bulk dma_scatter_add pushes them into the zero-initialized
     outputs_dropped at packed group j (add-to-zero == write)
  3. 4 indirect-DMA copy-scatters write mask_embedding x4 groups into
     the dropped positions of outputs_masked
  4. 4 indirect-DMA copy-scatters write the gathered groups into the
     kept positions of outputs_masked

(3) and (4) write disjoint groups and together cover every group, so
outputs_masked never depends on buffer zero-init and nothing serializes
on a blanket fill. Per-core counts are data-dependent; the single SPMD
program takes the real token count in a register (loaded from an input
tensor), pads the int16 token lists with -1 (consumed only up to the
count) and pads the int32 scatter lists with a sentinel the DMA bounds
check skips.
"""

import numpy as np

import concourse.bacc as bacc
import concourse.bass as bass
import concourse.mybir as mybir
from concourse.bass_utils import run_bass_kernel_spmd

B, L, D = 4, 4096, 1024
NUM_KEEP = 1024
N_CORES = 8
LS = L // N_CORES            # 512 length groups per core
G = B * D                    # 4096 elements per group (16KB)
R = LS * B                   # 2048 rows of D per core
GCOLS = LS // 128            # 4 token columns
W16 = LS // 16               # 32 int16 index columns per list
SENT = np.int32(8192)        # int32 OOB sentinel (> LS-1 -> bounds-check skip)

F32 = mybir.dt.float32
I32 = mybir.dt.int32
I16 = mybir.dt.int16

_BUILT = None


def _build():
    """One SPMD Bass program shared by all 8 cores."""
    nc = bacc.Bacc(None)
    # x is length-major: row l*B+b of [R, D] is inputs[b, lo+l, :]
    x = nc.dram_tensor("x", [R, D], F32, kind="ExternalInput")
    meb4 = nc.dram_tensor("meb4", [128, G], F32, kind="ExternalInput")
    # int16 token lists (wrapped [16, LS/16], replicated to 128 partitions):
    # cols [0:W16] = gather src groups, [W16:2*W16] = od dst groups
    idx16 = nc.dram_tensor("idx16", [128, 2 * W16], I16, kind="ExternalInput")
    # int32 group lists for indirect scatters, [p, c] = token c*128+p:
    # cols [0:GCOLS] = om kept dst, [GCOLS:2*GCOLS] = om dropped dst;
    # col [2*GCOLS] = token count (int32)
    idx32 = nc.dram_tensor("idx32", [128, 2 * GCOLS + 1], I32,
                           kind="ExternalInput")
    om = nc.dram_tensor("om", [R, D], F32, kind="ExternalOutput")
    od = nc.dram_tensor("od", [R + B, D], F32, kind="ExternalOutput")

    with (
        nc.Block() as block,
        nc.semaphore("ld") as ld,
        nc.semaphore("g0") as g0,
        nc.semaphore("sc") as sc,
        nc.sbuf_tensor("xk", [128, GCOLS, G], F32) as xk,
        nc.sbuf_tensor("mesb", [128, G], F32) as mesb,
        nc.sbuf_tensor("i16sb", [128, 2 * W16], I16) as i16sb,
        nc.sbuf_tensor("i32sb", [128, 2 * GCOLS + 1], I32) as i32sb,
    ):
        x_g = x[:].rearrange("(l b) d -> l (b d)", b=B)     # [LS, G]
        om_g = om[:].rearrange("(l b) d -> l (b d)", b=B)   # [LS, G]
        od_g = od[:].rearrange("(g b) d -> g (b d)", b=B)   # [LS+1, G]

        @block.sync
        def _(sync):
            sync.dma_start(out=i16sb[:], in_=idx16[:]).then_inc(ld, 16)
            sync.dma_start(out=i32sb[:], in_=idx32[:]).then_inc(ld, 16)
            sync.dma_start(out=mesb[:], in_=meb4[:]).then_inc(ld, 16)

        @block.gpsimd
        def _(g):
            g.wait_ge(ld, 48)
            with g.register("rk") as rk, g.register("bc") as bc:
                g.reg_load(rk, i32sb[:1, 2 * GCOLS:2 * GCOLS + 1])
                g.reg_mov(bc, LS - 1)
                g.dma_gather(
                    out_ap=xk[:],
                    in_ap=x_g,
                    idxs_ap=i16sb[:, 0:W16],
                    num_idxs=LS,
                    num_idxs_reg=rk,
                    elem_size=G,
                ).then_inc(g0, 16)
                # mask_embedding scatters are independent of the gather;
                # their prep overlaps the gather's transfer
                for c in range(GCOLS):
                    g.indirect_dma_start(
                        out=om_g,
                        out_offset=bass.IndirectOffsetOnAxis(
                            ap=i32sb[:, GCOLS + c:GCOLS + c + 1], axis=0),
                        in_=mesb[:],
                        in_offset=None,
                        bounds_check=bc,
                        oob_is_err=False,
                    ).then_inc(sc, 16)
                g.wait_ge(g0, 16)
                g.dma_scatter_add(
                    out_ap=od_g,
                    in_ap=xk[:],
                    idxs_ap=i16sb[:, W16:2 * W16],
                    num_idxs=LS,
                    num_idxs_reg=rk,
                    elem_size=G,
                ).then_inc(sc, 16)
                for c in range(GCOLS):
                    g.indirect_dma_start(
                        out=om_g,
                        out_offset=bass.IndirectOffsetOnAxis(
                            ap=i32sb[:, c:c + 1], axis=0),
                        in_=xk[:, c:c + 1, :].squeeze(1),
                        in_offset=None,
                        bounds_check=bc,
                        oob_is_err=False,
                    ).then_inc(sc, 16)
                g.wait_ge(sc, 16 * (2 * GCOLS + 1))

    nc.compile()
    return nc


def get_program():
    global _BUILT
    if _BUILT is None:
        _BUILT = _build()
    return _BUILT


def _wrap16(flat):
    """int16 token list -> [128, len/16] wrapped layout: token i at
    [i % 16, i // 16], replicated 8x along partitions."""
    arr = np.asarray(flat, dtype=np.int16).reshape(-1, 16).T  # [16, n/16]
    return np.tile(arr, (8, 1))


def make_core_inputs(inputs, mask_embedding, idx_keep):
    """Host-side sharding: length-major slice per core, token lists."""
    idx_keep = np.asarray(idx_keep)
    keep_starts = np.searchsorted(idx_keep, np.arange(0, L + LS, LS))
    me = np.asarray(mask_embedding, dtype=np.float32)
    meb4 = np.tile(me, (128, B)).astype(np.float32)   # [128, G]

    in_maps = []
    counts = []
    for c in range(N_CORES):
        lo, hi = c * LS, (c + 1) * LS
        kl = (idx_keep[keep_starts[c]:keep_starts[c + 1]] - lo).astype(np.int64)
        n_c = len(kl)
        drop_mask = np.ones(LS, dtype=bool)
        drop_mask[kl] = False
        dl = np.nonzero(drop_mask)[0]

        # token j (j-th kept group): gather src group kl[j], od dst group j
        src = np.full(LS, -1, dtype=np.int64)
        ddst = np.full(LS, -1, dtype=np.int64)
        src[:n_c] = kl
        ddst[:n_c] = np.arange(n_c)
        n_tok = n_c
        if n_c < LS:                   # dummy token keeps the count nonzero
            src[n_c] = 0
            ddst[n_c] = LS             # sacrificial od group
            n_tok = n_c + 1

        idx32 = np.full((128, 2 * GCOLS + 1), SENT, dtype=np.int32)
        jj = np.arange(n_c)
        idx32[jj % 128, jj // 128] = kl                   # om kept dst
        ii = np.arange(LS - n_c)
        idx32[ii % 128, GCOLS + ii // 128] = dl           # om dropped dst
        idx32[:, 2 * GCOLS] = n_tok

        # length-major layout: x_c[l*B+b] = inputs[b, lo+l]
        x_c = np.ascontiguousarray(
            inputs[:, lo:hi, :].transpose(1, 0, 2), dtype=np.float32
        ).reshape(R, D)
        in_maps.append({
            "x": x_c,
            "meb4": meb4,
            "idx16": np.concatenate([_wrap16(src), _wrap16(ddst)], axis=1),
            "idx32": idx32,
        })
        counts.append(n_c)
    return in_maps, counts, keep_starts


def kernel(inputs, mask_embedding, idx_keep):
    inputs = np.asarray(inputs)
    mask_embedding = np.asarray(mask_embedding)
    idx_keep = np.asarray(idx_keep).astype(np.int32)

    nc = get_program()
    in_maps, counts, keep_starts = make_core_inputs(
        inputs, mask_embedding, idx_keep)

    res = run_bass_kernel_spmd(nc, in_maps, list(range(N_CORES)))

    outputs_masked = np.empty((B, L, D), dtype=np.float32)
    outputs_dropped = np.empty((B, NUM_KEEP, D), dtype=np.float32)
    for c in range(N_CORES):
        lo, hi = c * LS, (c + 1) * LS
        outputs_masked[:, lo:hi, :] = (
            res.results[c]["om"].reshape(LS, B, D).transpose(1, 0, 2))
        n_c = counts[c]
        k0 = keep_starts[c]
        outputs_dropped[:, k0:k0 + n_c, :] = (
            res.results[c]["od"][:R].reshape(LS, B, D)[:n_c].transpose(1, 0, 2))

    mask_drop = np.ones(L, dtype=np.float32)
    mask_drop[idx_keep] = 0.0

    return outputs_dropped, outputs_masked, mask_drop, idx_keep


# revision 12
# speedup vs baseline: 1.6226x; 1.3915x over previous
"""Trainium2 Bass kernel for nn_MaskToken (scatter_memory).

Reference semantics (B=4, L=4096, D=1024, NUM_KEEP=1024):
  outputs_dropped[b, k, :] = inputs[b, idx_keep[k], :]          # gather
  outputs_masked[b, l, :]  = inputs[b, l, :] if l in idx_keep else mask_embedding
  mask_drop[l]             = 0.0 if l in idx_keep else 1.0
  idx_keep passthrough.

Strategy: shard the length axis across 8 cores (512 positions/core).
Each core's slice is laid out length-major ([512, B*D] = 512 groups of
16KB), so one DMA descriptor moves a length position for all 4 batches
at once — 4x fewer descriptors than row-granular movement, which is
what the Q7 descriptor-generation cost scales with. idx_keep is sorted,
so the kept positions of core c map to a contiguous span of
outputs_dropped — no collective needed. The device program is pure
indirect-DMA data movement (INDIRECT1D, mainline SWDGE ucode — no Q7
library load):

  1. 4 indirect gathers pull the kept groups (HBM -> SBUF), 128
     groups per instruction
  2. 4 indirect scatters write mask_embedding x4 into the dropped
     positions of outputs_masked (independent of the gathers; their
     descriptor prep and transfers overlap the gathers')
  3. per 128-token chunk, once its gather lands: one indirect scatter
     into packed outputs_dropped and one into the kept positions of
     outputs_masked

(2) and (3) write disjoint groups and together cover every group of
outputs_masked. Per-core counts are data-dependent; index lists are
padded with a sentinel that the DMA bounds check silently skips, so one
fixed SPMD program serves all cores.
"""

import numpy as np

import concourse.bacc as bacc
import concourse.bass as bass
import concourse.mybir as mybir
from concourse.bass_utils import run_bass_kernel_spmd

B, L, D = 4, 4096, 1024
NUM_KEEP = 1024
N_CORES = 8
LS = L // N_CORES            # 512 length groups per core
G = B * D                    # 4096 elements per group (16KB)
R = LS * B                   # 2048 rows of D per core
GC = LS // 128               # 4 token chunks of 128
SENT = np.int32(8192)        # OOB sentinel (> LS-1 -> bounds-check skip)

F32 = mybir.dt.float32
I32 = mybir.dt.int32

_BUILT = None


def _build():
    """One SPMD Bass program shared by all 8 cores."""
    nc = bacc.Bacc(None)
    # x is length-major: row l*B+b of [R, D] is inputs[b, lo+l, :]
    x = nc.dram_tensor("x", [R, D], F32, kind="ExternalInput")
    meb4 = nc.dram_tensor("meb4", [128, G], F32, kind="ExternalInput")
    # idx32[p, c]: token j=c*128+p of each list; SENT-padded.
    # cols [0:GC] = kept groups (gather src == om kept dst),
    # cols [GC:2*GC] = od dst (packed j), cols [2*GC:3*GC] = om dropped dst
    idx32 = nc.dram_tensor("idx32", [128, 3 * GC], I32, kind="ExternalInput")
    om = nc.dram_tensor("om", [R, D], F32, kind="ExternalOutput")
    od = nc.dram_tensor("od", [R, D], F32, kind="ExternalOutput")

    with (
        nc.Block() as block,
        nc.semaphore("ld") as ld,
        nc.semaphore("g0") as g0,
        nc.semaphore("g1") as g1,
        nc.semaphore("g2") as g2,
        nc.semaphore("g3") as g3,
        nc.semaphore("sc") as sc,
        nc.sbuf_tensor("xk", [128, GC, G], F32) as xk,
        nc.sbuf_tensor("mesb", [128, G], F32) as mesb,
        nc.sbuf_tensor("i32sb", [128, 3 * GC], I32) as i32sb,
    ):
        gsem = [g0, g1, g2, g3]
        x_g = x[:].rearrange("(l b) d -> l (b d)", b=B)     # [LS, G]
        om_g = om[:].rearrange("(l b) d -> l (b d)", b=B)   # [LS, G]
        od_g = od[:].rearrange("(g b) d -> g (b d)", b=B)   # [LS, G]

        @block.sync
        def _(sync):
            sync.dma_start(out=i32sb[:], in_=idx32[:]).then_inc(ld, 16)
            sync.dma_start(out=mesb[:], in_=meb4[:]).then_inc(ld, 16)

        @block.gpsimd
        def _(g):
            g.wait_ge(ld, 32)
            with g.register("bc") as bc:
                g.reg_mov(bc, LS - 1)
                for c in range(GC):
                    g.indirect_dma_start(
                        out=xk[:, c:c + 1, :].squeeze(1),
                        out_offset=None,
                        in_=x_g,
                        in_offset=bass.IndirectOffsetOnAxis(
                            ap=i32sb[:, c:c + 1], axis=0),
                        bounds_check=bc,
                        oob_is_err=False,
                    ).then_inc(gsem[c], 16)
                for c in range(GC):
                    g.indirect_dma_start(
                        out=om_g,
                        out_offset=bass.IndirectOffsetOnAxis(
                            ap=i32sb[:, 2 * GC + c:2 * GC + c + 1], axis=0),
                        in_=mesb[:],
                        in_offset=None,
                        bounds_check=bc,
                        oob_is_err=False,
                    ).then_inc(sc, 16)
                for c in range(GC):
                    g.wait_ge(gsem[c], 16)
                    g.indirect_dma_start(
                        out=od_g,
                        out_offset=bass.IndirectOffsetOnAxis(
                            ap=i32sb[:, GC + c:GC + c + 1], axis=0),
                        in_=xk[:, c:c + 1, :].squeeze(1),
                        in_offset=None,
                        bounds_check=bc,
                        oob_is_err=False,
                    ).then_inc(sc, 16)
                    g.indirect_dma_start(
                        out=om_g,
                        out_offset=bass.IndirectOffsetOnAxis(
                            ap=i32sb[:, c:c + 1], axis=0),
                        in_=xk[:, c:c + 1, :].squeeze(1),
                        in_offset=None,
                        bounds_check=bc,
                        oob_is_err=False,
                    ).then_inc(sc, 16)
                g.wait_ge(sc, 16 * 3 * GC)

    nc.compile()
    return nc


def get_program():
    global _BUILT
    if _BUILT is None:
        _BUILT = _build()
    return _BUILT


def make_core_inputs(inputs, mask_embedding, idx_keep):
    """Host-side sharding: length-major slice per core, index lists."""
    idx_keep = np.asarray(idx_keep)
    keep_starts = np.searchsorted(idx_keep, np.arange(0, L + LS, LS))
    me = np.asarray(mask_embedding, dtype=np.float32)
    meb4 = np.tile(me, (128, B)).astype(np.float32)   # [128, G]

    in_maps = []
    counts = []
    for c in range(N_CORES):
        lo, hi = c * LS, (c + 1) * LS
        kl = (idx_keep[keep_starts[c]:keep_starts[c + 1]] - lo).astype(np.int64)
        n_c = len(kl)
        drop_mask = np.ones(LS, dtype=bool)
        drop_mask[kl] = False
        dl = np.nonzero(drop_mask)[0]

        idx32 = np.full((128, 3 * GC), SENT, dtype=np.int32)
        jj = np.arange(n_c)
        idx32[jj % 128, jj // 128] = kl                 # kept: src == om dst
        idx32[jj % 128, GC + jj // 128] = jj            # od dst (packed)
        ii = np.arange(LS - n_c)
        idx32[ii % 128, 2 * GC + ii // 128] = dl        # om dropped dst

        # length-major layout: x_c[l*B+b] = inputs[b, lo+l]
        x_c = np.ascontiguousarray(
            inputs[:, lo:hi, :].transpose(1, 0, 2), dtype=np.float32
        ).reshape(R, D)
        in_maps.append({"x": x_c, "meb4": meb4, "idx32": idx32})
        counts.append(n_c)
    return in_maps, counts, keep_starts


def kernel(inputs, mask_embedding, idx_keep):
    inputs = np.asarray(inputs)
    mask_embedding = np.asarray(mask_embedding)
    idx_keep = np.asarray(idx_keep).astype(np.int32)

    nc = get_program()
    in_maps, counts, keep_starts = make_core_inputs(
        inputs, mask_embedding, idx_keep)

    res = run_bass_kernel_spmd(nc, in_maps, list(range(N_CORES)))

    outputs_masked = np.empty((B, L, D), dtype=np.float32)
    outputs_dropped = np.empty((B, NUM_KEEP, D), dtype=np.float32)
    for c in range(N_CORES):
        lo, hi = c * LS, (c + 1) * LS
        outputs_masked[:, lo:hi, :] = (
            res.results[c]["om"].reshape(LS, B, D).transpose(1, 0, 2))
        n_c = counts[c]
        k0 = keep_starts[c]
        outputs_dropped[:, k0:k0 + n_c, :] = (
            res.results[c]["od"].reshape(LS, B, D)[:n_c].transpose(1, 0, 2))

    mask_drop = np.ones(L, dtype=np.float32)
    mask_drop[idx_keep] = 0.0

    return outputs_dropped, outputs_masked, mask_drop, idx_keep


# revision 15
# speedup vs baseline: 1.7765x; 1.0949x over previous
"""Trainium2 Bass kernel for nn_MaskToken (scatter_memory).

Reference semantics (B=4, L=4096, D=1024, NUM_KEEP=1024):
  outputs_dropped[b, k, :] = inputs[b, idx_keep[k], :]          # gather
  outputs_masked[b, l, :]  = inputs[b, l, :] if l in idx_keep else mask_embedding
  mask_drop[l]             = 0.0 if l in idx_keep else 1.0
  idx_keep passthrough.

Strategy: shard the length axis across 8 cores (512 positions/core).
Each core's slice is laid out length-major ([512, B*D] = 512 groups of
16KB), so one DMA descriptor moves a length position for all 4 batches
at once — 4x fewer descriptors than row-granular movement, which is
what the Q7 descriptor-generation cost scales with. idx_keep is sorted,
so the kept positions of core c map to a contiguous span of
outputs_dropped — no collective needed. The device program is pure
indirect-DMA data movement (INDIRECT1D, mainline SWDGE ucode — no Q7
library load):

  1. 4 indirect gathers pull the kept groups (HBM -> SBUF), 128
     groups per instruction
  2. 4 indirect scatters write mask_embedding x4 into the dropped
     positions of outputs_masked (independent of the gathers; their
     descriptor prep and transfers overlap the gathers')
  3. per 128-token chunk, once its gather lands: one indirect scatter
     into packed outputs_dropped and one into the kept positions of
     outputs_masked

(2) and (3) write disjoint groups and together cover every group of
outputs_masked. Per-core counts are data-dependent; index lists are
padded with a sentinel that the DMA bounds check silently skips, so one
fixed SPMD program serves all cores.
"""

import numpy as np

import concourse.bacc as bacc
import concourse.bass as bass
import concourse.mybir as mybir
from concourse.bass_utils import run_bass_kernel_spmd

B, L, D = 4, 4096, 1024
NUM_KEEP = 1024
N_CORES = 8
LS = L // N_CORES            # 512 length groups per core
G = B * D                    # 4096 elements per group (16KB)
R = LS * B                   # 2048 rows of D per core
GC = LS // 128               # 4 token chunks of 128
SENT = np.int32(8192)        # OOB sentinel (> LS-1 -> bounds-check skip)

F32 = mybir.dt.float32
I32 = mybir.dt.int32

_BUILT = None


def _build():
    """One SPMD Bass program shared by all 8 cores."""
    nc = bacc.Bacc(None)
    # x is length-major: row l*B+b of [R, D] is inputs[b, lo+l, :]
    x = nc.dram_tensor("x", [R, D], F32, kind="ExternalInput")
    meb4 = nc.dram_tensor("meb4", [128, G], F32, kind="ExternalInput")
    # idx32[p, c]: token j=c*128+p of each list; SENT-padded.
    # cols [0:GC] = kept groups (gather src == om kept dst),
    # cols [GC:2*GC] = od dst (packed j), cols [2*GC:3*GC] = om dropped dst
    idx32 = nc.dram_tensor("idx32", [128, 3 * GC], I32, kind="ExternalInput")
    om = nc.dram_tensor("om", [R, D], F32, kind="ExternalOutput")
    od = nc.dram_tensor("od", [R, D], F32, kind="ExternalOutput")

    with (
        nc.Block() as block,
        nc.semaphore("ldi") as ldi,
        nc.semaphore("ldm") as ldm,
        nc.semaphore("g0") as g0,
        nc.semaphore("g1") as g1,
        nc.semaphore("g2") as g2,
        nc.semaphore("g3") as g3,
        nc.semaphore("sc") as sc,
        nc.sbuf_tensor("xk", [128, GC, G], F32) as xk,
        nc.sbuf_tensor("mesb", [128, G], F32) as mesb,
        nc.sbuf_tensor("i32sb", [128, 3 * GC], I32) as i32sb,
    ):
        gsem = [g0, g1, g2, g3]
        x_g = x[:].rearrange("(l b) d -> l (b d)", b=B)     # [LS, G]
        om_g = om[:].rearrange("(l b) d -> l (b d)", b=B)   # [LS, G]
        od_g = od[:].rearrange("(g b) d -> g (b d)", b=B)   # [LS, G]

        @block.sync
        def _(sync):
            sync.dma_start(out=i32sb[:], in_=idx32[:]).then_inc(ldi, 16)
            sync.dma_start(out=mesb[:], in_=meb4[:]).then_inc(ldm, 16)

        @block.gpsimd
        def _(g):
            g.wait_ge(ldi, 16)
            with g.register("bc") as bc:
                g.reg_mov(bc, LS - 1)
                for c in range(GC):
                    g.indirect_dma_start(
                        out=xk[:, c:c + 1, :].squeeze(1),
                        out_offset=None,
                        in_=x_g,
                        in_offset=bass.IndirectOffsetOnAxis(
                            ap=i32sb[:, c:c + 1], axis=0),
                        bounds_check=bc,
                        oob_is_err=False,
                    ).then_inc(gsem[c], 16)
                g.wait_ge(ldm, 16)
                for c in range(GC):
                    g.indirect_dma_start(
                        out=om_g,
                        out_offset=bass.IndirectOffsetOnAxis(
                            ap=i32sb[:, 2 * GC + c:2 * GC + c + 1], axis=0),
                        in_=mesb[:],
                        in_offset=None,
                        bounds_check=bc,
                        oob_is_err=False,
                    ).then_inc(sc, 16)
                for c in range(GC):
                    g.wait_ge(gsem[c], 16)
                    g.indirect_dma_start(
                        out=od_g,
                        out_offset=bass.IndirectOffsetOnAxis(
                            ap=i32sb[:, GC + c:GC + c + 1], axis=0),
                        in_=xk[:, c:c + 1, :].squeeze(1),
                        in_offset=None,
                        bounds_check=bc,
                        oob_is_err=False,
                    ).then_inc(sc, 16)
                    g.indirect_dma_start(
                        out=om_g,
                        out_offset=bass.IndirectOffsetOnAxis(
                            ap=i32sb[:, c:c + 1], axis=0),
                        in_=xk[:, c:c + 1, :].squeeze(1),
                        in_offset=None,
                        bounds_check=bc,
                        oob_is_err=False,
                    ).then_inc(sc, 16)
                g.wait_ge(sc, 16 * 3 * GC)

    nc.compile()
    return nc


def get_program():
    global _BUILT
    if _BUILT is None:
        _BUILT = _build()
    return _BUILT


def make_core_inputs(inputs, mask_embedding, idx_keep):
    """Host-side sharding: length-major slice per core, index lists."""
    idx_keep = np.asarray(idx_keep)
    keep_starts = np.searchsorted(idx_keep, np.arange(0, L + LS, LS))
    me = np.asarray(mask_embedding, dtype=np.float32)
    meb4 = np.tile(me, (128, B)).astype(np.float32)   # [128, G]

    in_maps = []
    counts = []
    for c in range(N_CORES):
        lo, hi = c * LS, (c + 1) * LS
        kl = (idx_keep[keep_starts[c]:keep_starts[c + 1]] - lo).astype(np.int64)
        n_c = len(kl)
        drop_mask = np.ones(LS, dtype=bool)
        drop_mask[kl] = False
        dl = np.nonzero(drop_mask)[0]

        idx32 = np.full((128, 3 * GC), SENT, dtype=np.int32)
        jj = np.arange(n_c)
        idx32[jj % 128, jj // 128] = kl                 # kept: src == om dst
        idx32[jj % 128, GC + jj // 128] = jj            # od dst (packed)
        ii = np.arange(LS - n_c)
        idx32[ii % 128, 2 * GC + ii // 128] = dl        # om dropped dst

        # length-major layout: x_c[l*B+b] = inputs[b, lo+l]
        x_c = np.ascontiguousarray(
            inputs[:, lo:hi, :].transpose(1, 0, 2), dtype=np.float32
        ).reshape(R, D)
        in_maps.append({"x": x_c, "meb4": meb4, "idx32": idx32})
        counts.append(n_c)
    return in_maps, counts, keep_starts


def kernel(inputs, mask_embedding, idx_keep):
    inputs = np.asarray(inputs)
    mask_embedding = np.asarray(mask_embedding)
    idx_keep = np.asarray(idx_keep).astype(np.int32)

    nc = get_program()
    in_maps, counts, keep_starts = make_core_inputs(
        inputs, mask_embedding, idx_keep)

    res = run_bass_kernel_spmd(nc, in_maps, list(range(N_CORES)))

    outputs_masked = np.empty((B, L, D), dtype=np.float32)
    outputs_dropped = np.empty((B, NUM_KEEP, D), dtype=np.float32)
    for c in range(N_CORES):
        lo, hi = c * LS, (c + 1) * LS
        outputs_masked[:, lo:hi, :] = (
            res.results[c]["om"].reshape(LS, B, D).transpose(1, 0, 2))
        n_c = counts[c]
        k0 = keep_starts[c]
        outputs_dropped[:, k0:k0 + n_c, :] = (
            res.results[c]["od"].reshape(LS, B, D)[:n_c].transpose(1, 0, 2))

    mask_drop = np.ones(L, dtype=np.float32)
    mask_drop[idx_keep] = 0.0

    return outputs_dropped, outputs_masked, mask_drop, idx_keep
